# revision 2
# baseline (speedup 1.0000x reference)
"""HDGC-style GNN message passing on 8 NeuronCores.

Data-parallel over N (per sharding hint): each core gets N/8 batches plus the
full (tiny) adjacencies and 1x1-conv weights, computes the fused block, and
results are gathered on host. Falls back to a pure-numpy implementation if
device execution is unavailable.
"""

import numpy as np

N, C, T, V, H, O = 64, 256, 64, 25, 3, 256
D = 32
BN_EPS = 1e-5
NCORES = 8

_CACHE = {}


def _forward(xp, x, A_prior, A_2hop, beta, lam, W_phi, b_phi, W_psi, b_psi,
             W_d, b_d, bn_gamma, bn_beta, bn_mean, bn_var, W_g, b_g):
    """Reference math, xp = numpy | jax.numpy. x: [n, C, T, V] (n = shard)."""
    n, c, t, v = x.shape
    h, d = H, D
    scale = d ** -0.5

    def conv1x1_heads(W, b):
        y = xp.einsum('nctv,ec->netv', x, W) + b[None, :, None, None]
        return (y.reshape(n, h, d, t, v)
                 .transpose(0, 3, 1, 4, 2)
                 .reshape(n * t, h, v, d))

    phi = conv1x1_heads(W_phi, b_phi)
    psi = conv1x1_heads(W_psi, b_psi)
    logits = xp.einsum('bhvd,bhwd->bhvw', phi, psi) * scale
    m = logits.max(axis=-1, keepdims=True)
    e = xp.exp(logits - m)
    A_adapt = e / e.sum(axis=-1, keepdims=True)

    lam_c = xp.clip(lam, 0.0, 1.0)
    A_final = (A_prior + beta * A_2hop)[None] + lam_c * A_adapt

    feat = x.transpose(0, 2, 3, 1).reshape(n * t, v, c)
    z = xp.einsum('bhvw,bwc->bhvc', A_final, feat)
    out = xp.einsum('bhvc,hoc->bvo', z, W_d) + b_d.sum(axis=0)
    out = out.reshape(n, t, v, -1).transpose(0, 3, 1, 2)

    inv = 1.0 / xp.sqrt(bn_var + BN_EPS)
    out = ((out - bn_mean[None, :, None, None]) * (inv * bn_gamma)[None, :, None, None]
           + bn_beta[None, :, None, None])

    gate = 1.0 / (1.0 + xp.exp(-(xp.einsum('nctv,oc->notv', x, W_g)
                                 + b_g[None, :, None, None])))
    out = gate * out + x
    return xp.maximum(out, 0.0)


def _kernel_jax(inputs):
    import jax
    import jax.numpy as jnp
    from functools import partial

    devs = jax.devices()[:NCORES]
    if len(devs) < NCORES:
        raise RuntimeError("need 8 cores")

    if "pmapped" not in _CACHE:
        weight_names = ["A_prior", "A_2hop", "beta", "lam", "W_phi", "b_phi",
                        "W_psi", "b_psi", "W_d", "b_d", "bn_gamma", "bn_beta",
                        "bn_mean", "bn_var", "W_g", "b_g"]

        def fn(x, w):
            return _forward(jnp, x, *[w[k] for k in weight_names])

        _CACHE["pmapped"] = jax.pmap(fn, in_axes=(0, None), devices=devs)
        _CACHE["wnames"] = weight_names

    x = np.asarray(inputs["x"], np.float32).reshape(NCORES, N // NCORES, C, T, V)
    w = {k: np.asarray(inputs[k], np.float32) for k in _CACHE["wnames"]}
    out = _CACHE["pmapped"](x, w)
    out = np.asarray(out, np.float32).reshape(N, O, T, V)
    return out


def kernel(**inputs) -> np.ndarray:
    try:
        return _kernel_jax(inputs)
    except Exception:
        args = [np.asarray(inputs[k], np.float32) for k in
                ["x", "A_prior", "A_2hop", "beta", "lam", "W_phi", "b_phi",
                 "W_psi", "b_psi", "W_d", "b_d", "bn_gamma", "bn_beta",
                 "bn_mean", "bn_var", "W_g", "b_g"]]
        return np.asarray(_forward(np, *args), np.float32)
